# revision 13
# baseline (speedup 1.0000x reference)
"""Multi-head self-attention (B=4, C=256, H=W=48, NH=8) on 8 TRN2 NeuronCores.

Sharding: 8 shards = 4 batches x 2 query-halves (no collectives).
Each core: K,V projections for its batch over all S=2304 positions,
Q projection for its 1152-query half, attention for all 8 heads over
its query half, output projection + bias + residual for its disjoint
[256, 1152] output slice.

Kernel layout notes:
  - All matmuls run as float32r (full PE rate at free-dim >= 256). The
    BIR verifier requires f32r matmul inputs to be *produced rounded*
    by a compute op, so Q/K/V^T/exp tiles are written as f32r by their
    producing DVE/ACT ops, and x / weights get explicit DVE rounding
    copies after the DMA load.
  - Scores are computed transposed, [t, q], so the A@V matmul needs no
    transposes anywhere: lhsT = K[d, t] slice, rhs = Q[d, q] slice.
  - exp runs on ScalarE directly from PSUM into SBUF (the PSUM->SBUF
    move is fused into the activation); softmax max-subtraction is
    skipped (scores ~ N(0,1), no overflow risk in f32).
  - The softmax denominator comes free from the A@V matmul via a ones
    column appended to V^T (lhsT is [128, 33]; row 32 of the psum
    accumulates sum_t exp(s)).
  - Normalization: DVE reciprocal of the denom row (lane 32), bounce
    through a DRAM scratch row, DMA-broadcast it across 32 partitions
    (stride-0 partition APs are only legal from DRAM), DVE multiply
    (one PSUM operand only), then a DMA moves the result to its head
    slot in attnout (DVE cannot shift partitions; DMA can).
  - This container's walrus allows ONE sem wait per instruction; a
    post-scheduling pass splits multi-wait instructions into
    single-wait same-engine nops (see _TileContextP).
"""

import numpy as np

import concourse.bass as bass
import concourse.mybir as mybir
import concourse.tile as tile
from concourse.vector_clock import ScopedClock
from concourse.bass_utils import run_bass_kernel_spmd

B, C, HH, WW = 4, 256, 48, 48
S = HH * WW            # 2304
NH, HD = 8, 32
SCALE = HD ** -0.5
SQ = S // 2            # 1152 queries per core
QC = 384               # q-chunk (fits one PSUM bank in f32)
NQC = SQ // QC         # 3
NTT = S // 128         # 18 t-tiles
TG = 3                 # t-tiles per exp group
NTG = NTT // TG        # 6
CT = C // 128          # 2 channel tiles
QCB = 192              # attention q-chunk (pairs: 6 subtiles of 256 = 3 banks)
NJB = SQ // QCB        # 6

F32 = mybir.dt.float32
F32R = mybir.dt.float32r
BF16 = mybir.dt.bfloat16
AF = mybir.ActivationFunctionType
ALU = mybir.AluOpType

N_CORES = 8


class _TileContextP(tile.TileContext):
    """TileContext adapted to a walrus that allows 1 sem wait/instruction.

    After Tile scheduling, every instruction carrying N>1 sem waits is
    rewritten to keep its last wait; the other N-1 waits move onto
    fresh single-wait nops inserted just before it on the same engine
    (engines execute their stream in order, so blocking at the nop is
    equivalent). The kernel-tail drain is built the same way.
    """

    def _split_multi_waits(self):
        nc = self.nc
        for fn in nc.m.functions:
            for bb in fn.blocks:
                new_insts = []
                for inst in bb.instructions:
                    si = inst.sync_info
                    if si is not None and len(si.on_wait) > 1:
                        waits = list(si.on_wait)
                        for w in waits[:-1]:
                            nop = mybir.InstNoOp(
                                name=nc.get_next_instruction_name(),
                                engine=inst.engine,
                                ins=[], outs=[],
                                sync_info=mybir.SyncInfo(on_wait=[w], on_update=[]),
                                bass_nofuse=True,
                            )
                            nc.register_instruction(nop, overwrite=True)
                            new_insts.append(nop)
                        inst.sync_info = mybir.SyncInfo(
                            on_wait=[waits[-1]], on_update=list(si.on_update)
                        )
                    new_insts.append(inst)
                bb.instructions = new_insts

    def _drain_and_barrier(self, tick_clock, wait_clock):
        carrier = self.nc.sync.nop(nofuse=True)
        wait_clock.add_sem_waits(
            carrier.ins, ScopedClock({None: tick_clock.global_clock})
        )
        self.nc.sync.drain()
        self.nc.all_engine_barrier()
        assert self.sems is not None
        popped = self.nc._tile_sem_poison_stack.pop()
        assert popped is self._sem_poison
        self.nc.clear_and_free_semaphores(list(self.sems.allocated().values()))
        self.nc.all_engine_barrier()
        self._split_multi_waits()


def _build_nc():
    nc = bass.Bass()

    xf_d = nc.dram_tensor("xf", [C, S], F32, kind="ExternalInput")
    xq_d = nc.dram_tensor("xq", [C, SQ], F32, kind="ExternalInput")
    wqt_d = nc.dram_tensor("wqt", [C, C], F32, kind="ExternalInput")
    wkt_d = nc.dram_tensor("wkt", [C, C], F32, kind="ExternalInput")
    wvt_d = nc.dram_tensor("wvt", [C, C], F32, kind="ExternalInput")
    wot_d = nc.dram_tensor("wot", [C, C], F32, kind="ExternalInput")
    bqp_d = nc.dram_tensor("bqp", [128, CT], F32, kind="ExternalInput")
    bkp_d = nc.dram_tensor("bkp", [128, CT], F32, kind="ExternalInput")
    bop_d = nc.dram_tensor("bop", [128, CT], F32, kind="ExternalInput")
    bv_d = nc.dram_tensor("bv", [C], F32, kind="ExternalInput")
    out_d = nc.dram_tensor("out", [C, SQ], F32, kind="ExternalOutput")

    # attention/output q-chunks: 4x256 + 1x128
    JCH = [(0, 256), (256, 256), (512, 256), (768, 256), (1024, 128)]

    with _TileContextP(nc) as tc:
        with (
            tc.tile_pool(name="singles", bufs=1) as singles,
            tc.tile_pool(name="sbig", bufs=1) as sbig,
            tc.tile_pool(name="expsp", bufs=3) as expsp,
            tc.tile_pool(name="smallp", bufs=4) as smallp,
            tc.tile_pool(name="outp", bufs=3) as outp,
            tc.tile_pool(name="drp", bufs=4, space="DRAM") as drp,
        ):
            # ---- static loads + rounding/cast copies --------------------
            # QKV projections run in bf16 (their outputs feed bf16
            # attention anyway); the output projection stays fp32r.
            w_bf = {}
            for nm, d in (("wqt", wqt_d), ("wkt", wkt_d), ("wvt", wvt_d)):
                ld = singles.tile([128, CT, C], F32, tag=f"{nm}_ld")
                nc.sync.dma_start(out=ld, in_=d.rearrange("(t p) o -> p t o", p=128))
                rb = singles.tile([128, CT, C], BF16, tag=f"{nm}_bf")
                nc.vector.tensor_copy(out=rb, in_=ld)
                w_bf[nm] = rb
            wqt_sb, wkt_sb, wvt_sb = w_bf["wqt"], w_bf["wkt"], w_bf["wvt"]

            bqp_sb = singles.tile([128, CT], F32)
            bkp_sb = singles.tile([128, CT], F32)
            nc.sync.dma_start(out=bqp_sb, in_=bqp_d[:, :])
            nc.sync.dma_start(out=bkp_sb, in_=bkp_d[:, :])

            bv_sb = singles.tile([128, C], F32)
            bv_ap = bv_d[:]
            nc.gpsimd.dma_start(
                out=bv_sb,
                in_=bass.AP(
                    tensor=bv_ap.tensor, offset=bv_ap.offset,
                    ap=[[0, 128]] + [list(a) for a in bv_ap.ap],
                ),
            )

            x_ld = [sbig.tile([128, S], F32, tag=f"x_ld{t}", name=f"x_ld{t}") for t in range(CT)]
            x_bf = [sbig.tile([128, S], BF16, tag=f"x_bf{t}", name=f"x_bf{t}") for t in range(CT)]
            xr = xf_d.rearrange("(t p) s -> p t s", p=128)
            for t in range(CT):
                nc.sync.dma_start(out=x_ld[t], in_=xr[:, t, :])
                nc.vector.tensor_copy(out=x_bf[t], in_=x_ld[t])
            xq_ld = sbig.tile([128, CT, SQ], F32)
            nc.sync.dma_start(out=xq_ld, in_=xq_d.rearrange("(t p) s -> p t s", p=128))
            xq_bf = sbig.tile([128, CT, SQ], BF16)
            nc.vector.tensor_copy(out=xq_bf, in_=xq_ld)

            # split K/Q/V^T/attnout so phase B/C can start on partial data
            k_t = [sbig.tile([128, S], BF16, tag=f"k{t}", name=f"k{t}") for t in range(CT)]
            q_t = [sbig.tile([128, SQ], BF16, tag=f"q{t}", name=f"q{t}") for t in range(CT)]
            vt_p = [sbig.tile([128, NTT, 2, HD + 1], BF16, tag=f"vt{p}", name=f"vt{p}")
                    for p in range(NH // 2)]
            att_j = [sbig.tile([128, CT, 256], F32R, tag=f"att{i}", name=f"att{i}")
                     for i in range(len(JCH))]

            ones_f32 = singles.tile([128, NTT, 2], F32)
            nc.vector.memset(ones_f32, 1.0)
            for p in range(NH // 2):
                nc.vector.tensor_copy(out=vt_p[p][:, :, :, HD], in_=ones_f32)

            # ---- phase A: projections (bf16), ct0 first -----------------
            def q_proj(ot):
                for j in range(NQC):
                    ps = psA.tile([128, QC], F32, tag="proj")
                    for kt in range(CT):
                        nc.tensor.matmul(
                            ps,
                            lhsT=wqt_sb[:, kt, ot * 128:(ot + 1) * 128],
                            rhs=xq_bf[:, kt, j * QC:(j + 1) * QC],
                            start=(kt == 0), stop=(kt == CT - 1),
                        )
                    nc.vector.tensor_scalar(
                        out=q_t[ot][:, j * QC:(j + 1) * QC],
                        in0=ps, scalar1=bqp_sb[:, ot:ot + 1], scalar2=None,
                        op0=ALU.add,
                    )

            def k_proj(ot):
                for j in range(S // QC):
                    ps = psA.tile([128, QC], F32, tag="proj")
                    for kt in range(CT):
                        nc.tensor.matmul(
                            ps,
                            lhsT=wkt_sb[:, kt, ot * 128:(ot + 1) * 128],
                            rhs=x_bf[kt][:, j * QC:(j + 1) * QC],
                            start=(kt == 0), stop=(kt == CT - 1),
                        )
                    nc.vector.tensor_scalar(
                        out=k_t[ot][:, j * QC:(j + 1) * QC],
                        in0=ps, scalar1=bkp_sb[:, ot:ot + 1], scalar2=None,
                        op0=ALU.add,
                    )

            def v_proj():
                bvr = bv_sb.rearrange("p (hp w d) -> p hp w d", hp=NH // 2, d=HD)
                for st in range(NTT):
                    ps = psA.tile([128, C], F32, tag="projv")
                    for kt in range(CT):
                        nc.tensor.matmul(
                            ps,
                            lhsT=x_bf[kt][:, st * 128:(st + 1) * 128],
                            rhs=wvt_sb[:, kt, :],
                            start=(kt == 0), stop=(kt == CT - 1),
                        )
                    psr = ps.rearrange("p (hp w d) -> p hp w d", hp=NH // 2, d=HD)
                    for p in range(NH // 2):
                        nc.vector.tensor_tensor(
                            out=vt_p[p][:, st, :, 0:HD],
                            in0=psr[:, p], in1=bvr[:, p], op=ALU.add,
                        )

            with tc.tile_pool(name="psA", bufs=4, space="PSUM") as psA:
                q_proj(0)
                k_proj(0)
                v_proj()
                q_proj(1)
                k_proj(1)

            wot_ld = singles.tile([128, CT, C], F32, tag="wot_ld")
            nc.sync.dma_start(out=wot_ld, in_=wot_d.rearrange("(t p) o -> p t o", p=128))
            wot_sb = singles.tile([128, CT, C], F32R, tag="wot_rb")
            nc.vector.tensor_copy(out=wot_sb, in_=wot_ld)
            bop_sb = singles.tile([128, CT], F32)
            nc.sync.dma_start(out=bop_sb, in_=bop_d[:, :])

            # ---- phase B + C: attention, fused output projection --------
            # q-chunk outer; after all 4 head-pairs finish a chunk, its
            # output projection runs (reusing the "av" psum slots) while
            # the next chunk's attention proceeds.
            out_r = out_d.rearrange("(t p) q -> p t q", p=128)
            with (
                tc.tile_pool(name="scp", bufs=2, space="PSUM") as scp,
                tc.tile_pool(name="avp", bufs=2, space="PSUM") as avp,
            ):
                for jidx, (j0, ln) in enumerate(JCH):
                    js = slice(j0, j0 + ln)
                    att = att_j[jidx]
                    for hp in range(NH // 2):
                        ha, hb = 2 * hp, 2 * hp + 1
                        heads = ((ha // 4, 32 * (ha % 4)), (hb // 4, 32 * (hb % 4)))
                        av = avp.tile([128, 256], F32, tag="av")
                        for g in range(NTG):
                            sc = scp.tile([128, 2 * TG, 256], F32, tag="sc")
                            for tt in range(TG):
                                t0 = (g * TG + tt) * 128
                                for hi, (ct, co) in enumerate(heads):
                                    nc.tensor.matmul(
                                        sc[:, hi * TG + tt, 0:ln],
                                        lhsT=k_t[ct][co:co + HD, t0:t0 + 128],
                                        rhs=q_t[ct][co:co + HD, js],
                                        start=True, stop=True,
                                        tile_position=(co, 0),
                                    )
                            ex = expsp.tile([128, 2 * TG, 256], BF16, tag="ex")
                            nc.scalar.activation(
                                out=ex[:, :, 0:ln], in_=sc[:, :, 0:ln],
                                func=AF.Exp, scale=SCALE,
                            )
                            for tt in range(TG):
                                st = g * TG + tt
                                first = (g == 0 and tt == 0)
                                last = (g == NTG - 1 and tt == TG - 1)
                                for hi in range(2):
                                    nc.tensor.matmul(
                                        av[64 * hi:64 * hi + HD + 1, 0:ln],
                                        lhsT=vt_p[hp][:, st, hi, :],
                                        rhs=ex[:, hi * TG + tt, 0:ln],
                                        start=first, stop=last,
                                        tile_position=(0, 64 * hi),
                                        skip_group_check=True,
                                    )
                        # normalize: reciprocal of denom lanes (32, 96),
                        # DRAM-bounce broadcast, lane-aligned multiplies,
                        # DMA shift into att head slots.
                        rec = smallp.tile([97, 256], F32, tag="rec")
                        nc.vector.reciprocal(rec[HD:HD + 1, 0:ln], av[HD:HD + 1, 0:ln])
                        nc.vector.reciprocal(rec[96:97, 0:ln], av[96:97, 0:ln])
                        dscr = drp.tile([2, 256], F32, tag="dscr")
                        nc.sync.dma_start(out=dscr[0:1, 0:ln], in_=rec[HD:HD + 1, 0:ln])
                        nc.sync.dma_start(out=dscr[1:2, 0:ln], in_=rec[96:97, 0:ln])
                        bc = smallp.tile([96, 256], F32, tag="bc")
                        for hi in range(2):
                            dap = dscr[hi:hi + 1, 0:ln]
                            nc.gpsimd.dma_start(
                                out=bc[64 * hi:64 * hi + HD, 0:ln],
                                in_=bass.AP(
                                    tensor=dap.tensor, offset=dap.offset,
                                    ap=[[0, HD]] + [list(a) for a in dap.ap[1:]],
                                ),
                            )
                        nrm = smallp.tile([96, 256], F32R, tag="nrm")
                        for hi, (ct, co) in enumerate(heads):
                            nc.vector.tensor_tensor(
                                out=nrm[64 * hi:64 * hi + HD, 0:ln],
                                in0=av[64 * hi:64 * hi + HD, 0:ln],
                                in1=bc[64 * hi:64 * hi + HD, 0:ln], op=ALU.mult,
                            )
                            nc.sync.dma_start(
                                out=att[co:co + HD, ct, 0:ln],
                                in_=nrm[64 * hi:64 * hi + HD, 0:ln],
                            )
                    # output projection for this q-chunk (fp32r)
                    for ot in range(CT):
                        ps = avp.tile([128, 256], F32, tag="av")
                        for kt in range(CT):
                            nc.tensor.matmul(
                                ps[:, 0:ln],
                                lhsT=wot_sb[:, kt, ot * 128:(ot + 1) * 128],
                                rhs=att[:, kt, 0:ln],
                                start=(kt == 0), stop=(kt == CT - 1),
                            )
                        ob = outp.tile([128, 256], F32, tag="ob")
                        nc.vector.tensor_scalar(
                            out=ob[:, 0:ln], in0=ps[:, 0:ln],
                            scalar1=bop_sb[:, ot:ot + 1],
                            scalar2=None, op0=ALU.add,
                        )
                        nc.vector.tensor_tensor(
                            out=ob[:, 0:ln], in0=ob[:, 0:ln],
                            in1=xq_ld[:, ot, js], op=ALU.add,
                        )
                        nc.sync.dma_start(
                            out=out_r[:, ot, js], in_=ob[:, 0:ln],
                        )

    return nc


_NC = None
LAST_RESULTS = None
TRACE = False


def _get_nc():
    global _NC
    if _NC is None:
        _NC = _build_nc()
    return _NC


def kernel(x, Wq, bq, Wk, bk, Wv, bv, Wo, bo):
    global LAST_RESULTS
    x = np.ascontiguousarray(np.asarray(x, dtype=np.float32).reshape(B, C, S))
    wqt = np.ascontiguousarray(np.asarray(Wq, dtype=np.float32).T)
    wkt = np.ascontiguousarray(np.asarray(Wk, dtype=np.float32).T)
    wvt = np.ascontiguousarray(np.asarray(Wv, dtype=np.float32).T)
    wot = np.ascontiguousarray(np.asarray(Wo, dtype=np.float32).T)
    bqp = np.ascontiguousarray(np.asarray(bq, dtype=np.float32).reshape(CT, 128).T)
    bkp = np.ascontiguousarray(np.asarray(bk, dtype=np.float32).reshape(CT, 128).T)
    bop = np.ascontiguousarray(np.asarray(bo, dtype=np.float32).reshape(CT, 128).T)
    bvv = np.ascontiguousarray(np.asarray(bv, dtype=np.float32))

    in_maps = []
    for core in range(N_CORES):
        b, half = divmod(core, 2)
        qlo = half * SQ
        in_maps.append({
            "xf": x[b],
            "xq": np.ascontiguousarray(x[b][:, qlo:qlo + SQ]),
            "wqt": wqt, "wkt": wkt, "wvt": wvt, "wot": wot,
            "bqp": bqp, "bkp": bkp, "bop": bop, "bv": bvv,
        })

    res = run_bass_kernel_spmd(_get_nc(), in_maps, list(range(N_CORES)), trace=TRACE)
    LAST_RESULTS = res

    out = np.empty((B, C, S), dtype=np.float32)
    for core in range(N_CORES):
        b, half = divmod(core, 2)
        qlo = half * SQ
        out[b][:, qlo:qlo + SQ] = res.results[core]["out"]
    return out.reshape(B, C, HH, WW)


# revision 15
# speedup vs baseline: 1.1676x; 1.1676x over previous
"""Multi-head self-attention (B=4, C=256, H=W=48, NH=8) on 8 TRN2 NeuronCores.

Sharding: 8 shards = 4 batches x 2 query-halves (no collectives).
Each core: K,V projections for its batch over all S=2304 positions,
Q projection for its 1152-query half, attention for all 8 heads over
its query half, output projection + bias + residual for its disjoint
[256, 1152] output slice.

Kernel layout notes:
  - All matmuls run as float32r (full PE rate at free-dim >= 256). The
    BIR verifier requires f32r matmul inputs to be *produced rounded*
    by a compute op, so Q/K/V^T/exp tiles are written as f32r by their
    producing DVE/ACT ops, and x / weights get explicit DVE rounding
    copies after the DMA load.
  - Scores are computed transposed, [t, q], so the A@V matmul needs no
    transposes anywhere: lhsT = K[d, t] slice, rhs = Q[d, q] slice.
  - exp runs on ScalarE directly from PSUM into SBUF (the PSUM->SBUF
    move is fused into the activation); softmax max-subtraction is
    skipped (scores ~ N(0,1), no overflow risk in f32).
  - The softmax denominator comes free from the A@V matmul via a ones
    column appended to V^T (lhsT is [128, 33]; row 32 of the psum
    accumulates sum_t exp(s)).
  - Normalization: DVE reciprocal of the denom row (lane 32), bounce
    through a DRAM scratch row, DMA-broadcast it across 32 partitions
    (stride-0 partition APs are only legal from DRAM), DVE multiply
    (one PSUM operand only), then a DMA moves the result to its head
    slot in attnout (DVE cannot shift partitions; DMA can).
  - This container's walrus allows ONE sem wait per instruction; a
    post-scheduling pass splits multi-wait instructions into
    single-wait same-engine nops (see _TileContextP).
"""

import numpy as np

import concourse.bass as bass
import concourse.mybir as mybir
import concourse.tile as tile
from concourse.vector_clock import ScopedClock
from concourse.bass_utils import run_bass_kernel_spmd

B, C, HH, WW = 4, 256, 48, 48
S = HH * WW            # 2304
NH, HD = 8, 32
SCALE = HD ** -0.5
SQ = S // 2            # 1152 queries per core
QC = 384               # q-chunk (fits one PSUM bank in f32)
NQC = SQ // QC         # 3
NTT = S // 128         # 18 t-tiles
TG = 3                 # t-tiles per exp group
NTG = NTT // TG        # 6
CT = C // 128          # 2 channel tiles
QCB = 192              # attention q-chunk (pairs: 6 subtiles of 256 = 3 banks)
NJB = SQ // QCB        # 6

F32 = mybir.dt.float32
F32R = mybir.dt.float32r
BF16 = mybir.dt.bfloat16
AF = mybir.ActivationFunctionType
ALU = mybir.AluOpType

N_CORES = 8


class _TileContextP(tile.TileContext):
    """TileContext adapted to a walrus that allows 1 sem wait/instruction.

    After Tile scheduling, every instruction carrying N>1 sem waits is
    rewritten to keep its last wait; the other N-1 waits move onto
    fresh single-wait nops inserted just before it on the same engine
    (engines execute their stream in order, so blocking at the nop is
    equivalent). The kernel-tail drain is built the same way.
    """

    def _split_multi_waits(self):
        nc = self.nc
        for fn in nc.m.functions:
            for bb in fn.blocks:
                new_insts = []
                for inst in bb.instructions:
                    si = inst.sync_info
                    if si is not None and len(si.on_wait) > 1:
                        waits = list(si.on_wait)
                        for w in waits[:-1]:
                            nop = mybir.InstNoOp(
                                name=nc.get_next_instruction_name(),
                                engine=inst.engine,
                                ins=[], outs=[],
                                sync_info=mybir.SyncInfo(on_wait=[w], on_update=[]),
                                bass_nofuse=True,
                            )
                            nc.register_instruction(nop, overwrite=True)
                            new_insts.append(nop)
                        inst.sync_info = mybir.SyncInfo(
                            on_wait=[waits[-1]], on_update=list(si.on_update)
                        )
                    new_insts.append(inst)
                bb.instructions = new_insts

    def _drain_and_barrier(self, tick_clock, wait_clock):
        carrier = self.nc.sync.nop(nofuse=True)
        wait_clock.add_sem_waits(
            carrier.ins, ScopedClock({None: tick_clock.global_clock})
        )
        self.nc.sync.drain()
        self.nc.all_engine_barrier()
        assert self.sems is not None
        popped = self.nc._tile_sem_poison_stack.pop()
        assert popped is self._sem_poison
        self.nc.clear_and_free_semaphores(list(self.sems.allocated().values()))
        self.nc.all_engine_barrier()
        self._split_multi_waits()


def _build_nc():
    nc = bass.Bass()

    xf_d = nc.dram_tensor("xf", [C, S], F32, kind="ExternalInput")
    xq_d = nc.dram_tensor("xq", [C, SQ], F32, kind="ExternalInput")
    wqt_d = nc.dram_tensor("wqt", [C, C], F32, kind="ExternalInput")
    wkt_d = nc.dram_tensor("wkt", [C, C], F32, kind="ExternalInput")
    wvt_d = nc.dram_tensor("wvt", [C, C], F32, kind="ExternalInput")
    wot_d = nc.dram_tensor("wot", [C, C], F32, kind="ExternalInput")
    bqp_d = nc.dram_tensor("bqp", [128, CT], F32, kind="ExternalInput")
    bkp_d = nc.dram_tensor("bkp", [128, CT], F32, kind="ExternalInput")
    bop_d = nc.dram_tensor("bop", [128, CT], F32, kind="ExternalInput")
    bv_d = nc.dram_tensor("bv", [C], F32, kind="ExternalInput")
    out_d = nc.dram_tensor("out", [C, SQ], F32, kind="ExternalOutput")

    # attention/output q-chunks: 4x256 + 1x128
    JCH = [(0, 256), (256, 256), (512, 256), (768, 256), (1024, 128)]

    with _TileContextP(nc) as tc:
        with (
            tc.tile_pool(name="singles", bufs=1) as singles,
            tc.tile_pool(name="sbig", bufs=1) as sbig,
            tc.tile_pool(name="expsp", bufs=4) as expsp,
            tc.tile_pool(name="smallp", bufs=4) as smallp,
            tc.tile_pool(name="outp", bufs=3) as outp,
            tc.tile_pool(name="drp", bufs=4, space="DRAM") as drp,
        ):
            # ---- static loads + rounding/cast copies --------------------
            # QKV projections run in bf16 (their outputs feed bf16
            # attention anyway); the output projection stays fp32r.
            w_bf = {}
            for nm, d in (("wqt", wqt_d), ("wkt", wkt_d), ("wvt", wvt_d)):
                ld = singles.tile([128, CT, C], F32, tag=f"{nm}_ld")
                nc.sync.dma_start(out=ld, in_=d.rearrange("(t p) o -> p t o", p=128))
                rb = singles.tile([128, CT, C], BF16, tag=f"{nm}_bf")
                nc.vector.tensor_copy(out=rb, in_=ld)
                w_bf[nm] = rb
            wqt_sb, wkt_sb, wvt_sb = w_bf["wqt"], w_bf["wkt"], w_bf["wvt"]

            bqp_sb = singles.tile([128, CT], F32)
            bkp_sb = singles.tile([128, CT], F32)
            nc.sync.dma_start(out=bqp_sb, in_=bqp_d[:, :])
            nc.sync.dma_start(out=bkp_sb, in_=bkp_d[:, :])

            bv_sb = singles.tile([128, C], F32)
            bv_ap = bv_d[:]
            nc.gpsimd.dma_start(
                out=bv_sb,
                in_=bass.AP(
                    tensor=bv_ap.tensor, offset=bv_ap.offset,
                    ap=[[0, 128]] + [list(a) for a in bv_ap.ap],
                ),
            )

            x_ld = [sbig.tile([128, S], F32, tag=f"x_ld{t}", name=f"x_ld{t}") for t in range(CT)]
            x_bf = [sbig.tile([128, S], BF16, tag=f"x_bf{t}", name=f"x_bf{t}") for t in range(CT)]
            xr = xf_d.rearrange("(t p) s -> p t s", p=128)
            nc.sync.dma_start(out=x_ld[0], in_=xr[:, 0, :])
            nc.scalar.dma_start(out=x_ld[1], in_=xr[:, 1, :])
            for t in range(CT):
                nc.vector.tensor_copy(out=x_bf[t], in_=x_ld[t])
            xq_ld = sbig.tile([128, CT, SQ], F32)
            nc.gpsimd.dma_start(out=xq_ld, in_=xq_d.rearrange("(t p) s -> p t s", p=128))
            xq_bf = sbig.tile([128, CT, SQ], BF16)
            nc.vector.tensor_copy(out=xq_bf, in_=xq_ld)

            # split K/Q/V^T/attnout so phase B/C can start on partial data
            k_t = [sbig.tile([128, S], BF16, tag=f"k{t}", name=f"k{t}") for t in range(CT)]
            q_t = [sbig.tile([128, SQ], BF16, tag=f"q{t}", name=f"q{t}") for t in range(CT)]
            vt_p = [sbig.tile([128, NTT, 2, HD + 1], BF16, tag=f"vt{p}", name=f"vt{p}")
                    for p in range(NH // 2)]
            att_j = [sbig.tile([128, CT, 256], F32R, tag=f"att{i}", name=f"att{i}")
                     for i in range(len(JCH))]

            ones_f32 = singles.tile([128, NTT, 2], F32)
            nc.vector.memset(ones_f32, 1.0)
            for p in range(NH // 2):
                nc.vector.tensor_copy(out=vt_p[p][:, :, :, HD], in_=ones_f32)

            # ---- phase A: projections (bf16), ct=0 upfront; ct=1 is
            # emitted later as PE gap-filler during attention ------------
            def q_proj2(ot, psA):
                for j in range(NQC):
                    ps = psA.tile([128, QC], F32, tag="proj", name=f"qp{ot}{j}")
                    for kt in range(CT):
                        nc.tensor.matmul(
                            ps,
                            lhsT=wqt_sb[:, kt, ot * 128:(ot + 1) * 128],
                            rhs=xq_bf[:, kt, j * QC:(j + 1) * QC],
                            start=(kt == 0), stop=(kt == CT - 1),
                        )
                    nc.vector.tensor_scalar(
                        out=q_t[ot][:, j * QC:(j + 1) * QC],
                        in0=ps, scalar1=bqp_sb[:, ot:ot + 1], scalar2=None,
                        op0=ALU.add,
                    )

            def k_proj2(ot, psA):
                for j in range(S // QC):
                    ps = psA.tile([128, QC], F32, tag="proj", name=f"kp{ot}{j}")
                    for kt in range(CT):
                        nc.tensor.matmul(
                            ps,
                            lhsT=wkt_sb[:, kt, ot * 128:(ot + 1) * 128],
                            rhs=x_bf[kt][:, j * QC:(j + 1) * QC],
                            start=(kt == 0), stop=(kt == CT - 1),
                        )
                    nc.vector.tensor_scalar(
                        out=k_t[ot][:, j * QC:(j + 1) * QC],
                        in0=ps, scalar1=bkp_sb[:, ot:ot + 1], scalar2=None,
                        op0=ALU.add,
                    )

            def v_proj2(psA):
                bvr = bv_sb.rearrange("p (hp w d) -> p hp w d", hp=NH // 2, d=HD)
                for st in range(NTT):
                    ps = psA.tile([128, C], F32, tag="projv", name=f"vp{st}")
                    for kt in range(CT):
                        nc.tensor.matmul(
                            ps,
                            lhsT=x_bf[kt][:, st * 128:(st + 1) * 128],
                            rhs=wvt_sb[:, kt, :],
                            start=(kt == 0), stop=(kt == CT - 1),
                        )
                    psr = ps.rearrange("p (hp w d) -> p hp w d", hp=NH // 2, d=HD)
                    for p in range(NH // 2):
                        nc.vector.tensor_tensor(
                            out=vt_p[p][:, st, :, 0:HD],
                            in0=psr[:, p], in1=bvr[:, p], op=ALU.add,
                        )

            with tc.tile_pool(name="psA", bufs=4, space="PSUM") as psA:
                q_proj2(0, psA)
                k_proj2(0, psA)
                v_proj2(psA)
                q_proj2(1, psA)
                k_proj2(1, psA)

            wot_ld = singles.tile([128, CT, C], F32, tag="wot_ld")
            nc.sync.dma_start(out=wot_ld, in_=wot_d.rearrange("(t p) o -> p t o", p=128))
            wot_sb = singles.tile([128, CT, C], F32R, tag="wot_rb")
            nc.vector.tensor_copy(out=wot_sb, in_=wot_ld)
            bop_sb = singles.tile([128, CT], F32)
            nc.sync.dma_start(out=bop_sb, in_=bop_d[:, :])

            # ---- phase B: attention (+ phase C at the end) --------------
            # Emission order = Tile priority: attention for the ct=0
            # pairs is emitted right after their inputs (Q0/K0/V), and
            # the ct=1 projections AFTER them -- so the PE fills
            # exp-wait gaps with projection matmuls instead of blocking
            # the first exp behind all of phase A.
            def attention_pair(hp, scp, avp):
                ha, hb = 2 * hp, 2 * hp + 1
                heads = ((ha // 4, 32 * (ha % 4)), (hb // 4, 32 * (hb % 4)))
                for jidx, (j0, ln) in enumerate(JCH):
                    js = slice(j0, j0 + ln)
                    att = att_j[jidx]
                    av = avp.tile([128, 256], F32, tag="av", name=f"av{hp}{jidx}")
                    for g in range(NTG):
                        sc = scp.tile([128, 2 * TG, 256], F32, tag="sc",
                                      name=f"sc{hp}{jidx}{g}")
                        for tt in range(TG):
                            t0 = (g * TG + tt) * 128
                            for hi, (ct, co) in enumerate(heads):
                                nc.tensor.matmul(
                                    sc[:, hi * TG + tt, 0:ln],
                                    lhsT=k_t[ct][co:co + HD, t0:t0 + 128],
                                    rhs=q_t[ct][co:co + HD, js],
                                    start=True, stop=True,
                                    tile_position=(co, 0),
                                )
                        ex = expsp.tile([128, 2 * TG, 256], BF16, tag="ex",
                                        name=f"ex{hp}{jidx}{g}")
                        nc.scalar.activation(
                            out=ex[:, :, 0:ln], in_=sc[:, :, 0:ln],
                            func=AF.Exp, scale=SCALE,
                        )
                        for tt in range(TG):
                            st = g * TG + tt
                            first = (g == 0 and tt == 0)
                            last = (g == NTG - 1 and tt == TG - 1)
                            for hi in range(2):
                                nc.tensor.matmul(
                                    av[64 * hi:64 * hi + HD + 1, 0:ln],
                                    lhsT=vt_p[hp][:, st, hi, :],
                                    rhs=ex[:, hi * TG + tt, 0:ln],
                                    start=first, stop=last,
                                    tile_position=(0, 64 * hi),
                                    skip_group_check=True,
                                )
                    # normalize: reciprocal of denom lanes (32, 96),
                    # DRAM-bounce broadcast, lane-aligned multiplies,
                    # DMA shift into att head slots.
                    rec = smallp.tile([97, 256], F32, tag="rec",
                                      name=f"rec{hp}{jidx}")
                    nc.vector.reciprocal(rec[HD:HD + 1, 0:ln], av[HD:HD + 1, 0:ln])
                    nc.vector.reciprocal(rec[96:97, 0:ln], av[96:97, 0:ln])
                    dscr = drp.tile([2, 256], F32, tag="dscr",
                                    name=f"dscr{hp}{jidx}")
                    nc.sync.dma_start(out=dscr[0:1, 0:ln], in_=rec[HD:HD + 1, 0:ln])
                    nc.sync.dma_start(out=dscr[1:2, 0:ln], in_=rec[96:97, 0:ln])
                    bc = smallp.tile([96, 256], F32, tag="bc", name=f"bc{hp}{jidx}")
                    for hi in range(2):
                        dap = dscr[hi:hi + 1, 0:ln]
                        nc.gpsimd.dma_start(
                            out=bc[64 * hi:64 * hi + HD, 0:ln],
                            in_=bass.AP(
                                tensor=dap.tensor, offset=dap.offset,
                                ap=[[0, HD]] + [list(a) for a in dap.ap[1:]],
                            ),
                        )
                    nrm = smallp.tile([96, 256], F32R, tag="nrm",
                                      name=f"nrm{hp}{jidx}")
                    for hi, (ct, co) in enumerate(heads):
                        nc.vector.tensor_tensor(
                            out=nrm[64 * hi:64 * hi + HD, 0:ln],
                            in0=av[64 * hi:64 * hi + HD, 0:ln],
                            in1=bc[64 * hi:64 * hi + HD, 0:ln], op=ALU.mult,
                        )
                        nc.sync.dma_start(
                            out=att[co:co + HD, ct, 0:ln],
                            in_=nrm[64 * hi:64 * hi + HD, 0:ln],
                        )

            with (
                tc.tile_pool(name="scp", bufs=2, space="PSUM") as scp,
                tc.tile_pool(name="avp", bufs=2, space="PSUM") as avp,
            ):
                for hp in range(NH // 2):
                    attention_pair(hp, scp, avp)

            # ---- phase C: output projection + bias + residual (fp32r) ---
            out_r = out_d.rearrange("(t p) q -> p t q", p=128)
            with tc.tile_pool(name="psC", bufs=2, space="PSUM") as psC:
                for jidx, (j0, ln) in enumerate(JCH):
                    js = slice(j0, j0 + ln)
                    for ot in range(CT):
                        ps = psC.tile([128, 256], F32, tag="proj",
                                      name=f"cps{jidx}{ot}")
                        for kt in range(CT):
                            nc.tensor.matmul(
                                ps[:, 0:ln],
                                lhsT=wot_sb[:, kt, ot * 128:(ot + 1) * 128],
                                rhs=att_j[jidx][:, kt, 0:ln],
                                start=(kt == 0), stop=(kt == CT - 1),
                            )
                        ob = outp.tile([128, 256], F32, tag="ob",
                                       name=f"ob{jidx}{ot}")
                        nc.vector.tensor_scalar(
                            out=ob[:, 0:ln], in0=ps[:, 0:ln],
                            scalar1=bop_sb[:, ot:ot + 1],
                            scalar2=None, op0=ALU.add,
                        )
                        nc.vector.tensor_tensor(
                            out=ob[:, 0:ln], in0=ob[:, 0:ln],
                            in1=xq_ld[:, ot, js], op=ALU.add,
                        )
                        nc.sync.dma_start(
                            out=out_r[:, ot, js], in_=ob[:, 0:ln],
                        )

    return nc


_NC = None
LAST_RESULTS = None
TRACE = False


def _get_nc():
    global _NC
    if _NC is None:
        _NC = _build_nc()
    return _NC


def kernel(x, Wq, bq, Wk, bk, Wv, bv, Wo, bo):
    global LAST_RESULTS
    x = np.ascontiguousarray(np.asarray(x, dtype=np.float32).reshape(B, C, S))
    wqt = np.ascontiguousarray(np.asarray(Wq, dtype=np.float32).T)
    wkt = np.ascontiguousarray(np.asarray(Wk, dtype=np.float32).T)
    wvt = np.ascontiguousarray(np.asarray(Wv, dtype=np.float32).T)
    wot = np.ascontiguousarray(np.asarray(Wo, dtype=np.float32).T)
    bqp = np.ascontiguousarray(np.asarray(bq, dtype=np.float32).reshape(CT, 128).T)
    bkp = np.ascontiguousarray(np.asarray(bk, dtype=np.float32).reshape(CT, 128).T)
    bop = np.ascontiguousarray(np.asarray(bo, dtype=np.float32).reshape(CT, 128).T)
    bvv = np.ascontiguousarray(np.asarray(bv, dtype=np.float32))

    in_maps = []
    for core in range(N_CORES):
        b, half = divmod(core, 2)
        qlo = half * SQ
        in_maps.append({
            "xf": x[b],
            "xq": np.ascontiguousarray(x[b][:, qlo:qlo + SQ]),
            "wqt": wqt, "wkt": wkt, "wvt": wvt, "wot": wot,
            "bqp": bqp, "bkp": bkp, "bop": bop, "bv": bvv,
        })

    res = run_bass_kernel_spmd(_get_nc(), in_maps, list(range(N_CORES)), trace=TRACE)
    LAST_RESULTS = res

    out = np.empty((B, C, S), dtype=np.float32)
    for core in range(N_CORES):
        b, half = divmod(core, 2)
        qlo = half * SQ
        out[b][:, qlo:qlo + SQ] = res.results[core]["out"]
    return out.reshape(B, C, HH, WW)
